# revision 89
# baseline (speedup 1.0000x reference)
"""Trainium2 Bass kernel for nn_DynamicMLP (3-layer LIF spiking net, T=16).

Strategy (8 NeuronCores, data-parallel over batch):
  - Shard batch 1024 -> 8 x 128. Replicate weights. Zero cross-core comms.
  - Layout: [batch=128 partitions, hidden on free dim].
  - The LIF current state c lives ENTIRELY in PSUM, scaled by 2^t:
      C_t = sum_{tau<=t} 2^tau * I_tau  ==  2^t * c_t  (bitwise-equivalent to the
      reference's c = 0.5*c + I decay, since powers of 2 are exact).
    Inputs are pre-scaled by 2^t on host (x) / on device (spikes).
  - The output is chaotically sensitive (1e-6 current noise -> 2% output
    error), so matmuls must be fp32-exact. They run as fp16 multi-term splits
    (fp16 x fp16 products are exact in fp32 PSUM accumulation; all stored
    operands kept in fp16 normal range; ~1e-7 residual):
      L0: x = xh + xl exactly (fp16 pair). 2^t*xh@wh -> C0;
          2^(t+11)*xl@wh and 2^t*xh@(wl*2^11) -> C0b (folded at 2^-(t+11)).
      L1/L2: spikes s*2^t are fp16-exact; s_hi@wh -> C and
          (s_hi*2^-11)@(wl*2^11) -> C, same scale, no extra banks.
    Residual error ~2e-8 per current, inside the fp32 matmul-order envelope.
  - Spikes are emitted as fp16 * 2^t and DMA-transposed (xbar) to become the
    next layer's stationary operand. Biases enter via a K=2 matmul row pair
    (rows scaled 2^t and 2^(t-11) for the hi/lo bias split).
  - All DRAM operands are pre-arranged host-side to [128, free] partition-major
    blocks so every DMA is one large contiguous copy (>=512B runs, no 2x DMA
    penalty), and the DMA issue order is the startup schedule.
  - The output spike accumulator lives in SBUF and is summed on the idle Pool
    (gpsimd) engine, freeing PE columns and one PSUM bank.
"""
import sys

sys.path.insert(0, "/opt/trn_rl_repo")

import numpy as np

import concourse.bacc as bacc
import concourse.tile as tile
from concourse import mybir
from concourse.bass_utils import run_bass_kernel_spmd

dt = mybir.dt
F16 = dt.float16
F32 = dt.float32
Alu = mybir.AluOpType

NCORES = 8
FULL = dict(T=16, IN=2048, H0=1024, H1=1024, OUT=512, BL=128)
EXACT_ORDER = True  # reproduce the reference LIF rounding order exactly

_BUILD_CACHE = {}

# weight DMA group sizes (k-chunks per DMA/tile)
GK = {"w0": 2, "w1": 4, "wo": 8}


def build(T=16, IN=2048, H0=1024, H1=1024, OUT=512, BL=128):
    key = (T, IN, H0, H1, OUT, BL, EXACT_ORDER)
    if key in _BUILD_CACHE:
        return _BUILD_CACHE[key]
    KT0, KT1, KT2 = IN // 128, H0 // 128, H1 // 128
    NCH = 512  # psum bank free-dim (fp32)
    HS = {0: H0, 1: H1, 2: OUT}

    nc = bacc.Bacc("TRN2", target_bir_lowering=False, debug=False, num_devices=NCORES)

    xa_d = nc.dram_tensor("xa", [T, 128, KT0 * BL], F16, kind="ExternalInput")
    xr_d = nc.dram_tensor("xr", [T, 128, KT0 * BL], F16, kind="ExternalInput")
    w_d = {}
    for nm, (kt, h) in {"w0": (KT0, H0), "w1": (KT1, H1), "wo": (KT2, OUT)}.items():
        for sfx in ("a", "l"):
            w_d[nm + sfx] = nc.dram_tensor(nm + sfx, [128, kt * h], F16,
                                           kind="ExternalInput")
    b_d = {}
    for nm, h in {"b0": H0, "b1": H1, "b2": OUT}.items():
        b_d[nm] = nc.dram_tensor(nm, [2, h], F16, kind="ExternalInput")
    out_d = nc.dram_tensor("out", [BL, OUT], F32, kind="ExternalOutput")

    with tile.TileContext(nc) as tc:
        with tc.tile_pool(name="w", bufs=1) as wp, \
             tc.tile_pool(name="state", bufs=1) as sp, \
             tc.tile_pool(name="xs", bufs=2) as xp, \
             tc.tile_pool(name="spk", bufs=2) as kp, \
             tc.tile_pool(name="psum", bufs=1, space="PSUM") as pp:

            # ---- resident weights: one tile per DMA group ----
            w_sb = {}
            for nm, (kt, h) in {"w0": (KT0, H0), "w1": (KT1, H1),
                                "wo": (KT2, OUT)}.items():
                gk = GK[nm]
                for sfx in ("a", "l"):
                    w_sb[nm + sfx] = [
                        wp.tile([128, gk * h], F16, tag=f"{nm}{sfx}{g}",
                                name=f"{nm}{sfx}{g}")
                        for g in range(kt // gk)]

            def dma_w(nm, sfx, g):
                kt, h = {"w0": (KT0, H0), "w1": (KT1, H1), "wo": (KT2, OUT)}[nm]
                gk = GK[nm]
                nc.sync.dma_start(
                    out=w_sb[nm + sfx][g][:],
                    in_=w_d[nm + sfx][:, g * gk * h:(g + 1) * gk * h])

            def wsl(nm, sfx, kg, h, n0, nn):
                gk = GK[nm]
                tl = w_sb[nm + sfx][kg // gk]
                o = (kg % gk) * h + n0
                return tl[:, o:o + nn]

            b_sb = {}
            for nm, h in {"b0": H0, "b1": H1, "b2": OUT}.items():
                b_sb[nm] = wp.tile([2, h], F16, tag=nm, name=nm)

            def dma_b(nm):
                nc.sync.dma_start(out=b_sb[nm][:], in_=b_d[nm][:])

            # ---- states (single-buffered; DVE program order serializes) ----
            st = {}
            for l in (0, 1, 2):
                for nm in ("u0", "v0", "q"):
                    st[(l, nm)] = sp.tile([128, HS[l]], F32, tag=f"{nm}{l}",
                                          name=f"{nm}{l}")
            scrV = sp.tile([128, max(H0, H1)], F32, tag="scrV", name="scrV")
            scrB1 = sp.tile([128, H1], F32, tag="scrB1", name="scrB1")
            c021 = sp.tile([128, max(H0, H1)], F32, tag="c021")
            scrA = sp.tile([128, max(H0, H1)], F32, tag="scrA")
            scrA2 = sp.tile([128, max(H0, H1)], F32, tag="scrA2")
            scrB0b = sp.tile([128, H0], F32, tag="scrB0b", name="scrB0b")
            scrB0 = sp.tile([128, H0], F32, tag="scrB0", name="scrB0")
            accS = sp.tile([128, OUT], F32, tag="accS", name="accS")
            # psum current accumulators (2^t-scaled)
            C = {0: pp.tile([128, H0], F32, tag="C0", name="C0"),
                 1: pp.tile([128, H1], F32, tag="C1", name="C1"),
                 2: pp.tile([128, OUT], F32, tag="C2", name="C2")}
            C0b = pp.tile([128, H0], F32, tag="C0b", name="C0b")
            dummyP = pp.tile([128, NCH], F32, tag="dummyP", name="dummyP")

            def warm(n):
                """Keep the PE clock ramped through a known stall window:
                n independent throwaway matmuls into the spare PSUM bank."""
                for _ in range(n):
                    nc.tensor.matmul(dummyP[:], b_sb["b0"][:, :128],
                                     b_sb["b0"][:, :NCH], start=True, stop=True,
                                     skip_group_check=True)

            # ---- init ----
            for l in (0, 1, 2):
                for nm in ("u0", "v0", "q"):
                    nc.vector.memset(st[(l, nm)][:], 0.0)
            nc.vector.memset(c021[:], 0.021)
            nc.vector.memset(accS[:], 0.0)

            def lif_B(l, t):
                """Early-release C0/C0b into scratch on ACT so the next
                step's L0 matmuls can reuse the banks (2^-t scales exact).
                l=1/2 skip this: their chains read PSUM directly via stt."""
                assert l == 0
                nc.scalar.mul(scrB0[:], C[0][:], float(2.0 ** -t))
                nc.scalar.mul(scrB0b[:], C0b[:], float(2.0 ** -(t + 11)))

            def lif_ops(l, t, s_out, last=False, v_tile=None):
                """Emit LIF elementwise ops for layer l at step t.

                Consumes c_t (scrB0/scrB0b for l=0; direct 2^-t PSUM read for
                l=1/2), states v0/u0/q from step t-1. Produces v (=v_t),
                updates u0/v0/q for t+1, and the 2^t-scaled fp16 spikes.
                """
                h = HS[l]
                u0, v0, q = (st[(l, n)] for n in ("u0", "v0", "q"))
                v = (v_tile if v_tile is not None else scrV)[:, :h]
                A = scrA[:, :h]

                def add_c():
                    # v += c_t, reference rounding (2^-t scaling is exact)
                    if l == 0:
                        nc.vector.tensor_tensor(out=v, in0=v, in1=scrB0b[:],
                                                op=Alu.add)
                        nc.vector.tensor_tensor(out=v, in0=v, in1=scrB0[:],
                                                op=Alu.add)
                    else:
                        nc.vector.scalar_tensor_tensor(
                            out=v, in0=C[l][:], scalar=float(2.0 ** -t), in1=v,
                            op0=Alu.mult, op1=Alu.add)

                if last:
                    # final step: no state carry needed; short chain
                    nc.vector.tensor_tensor(out=v, in0=q[:], in1=v0[:],
                                            op=Alu.subtract)
                    nc.vector.tensor_tensor(out=v, in0=v, in1=u0[:],
                                            op=Alu.subtract)
                    add_c()
                    nc.vector.tensor_tensor(out=v, in0=v0[:], in1=v, op=Alu.add)
                    s_scale = 1.0 if l == 2 else float(2.0 ** t)
                    nc.vector.tensor_scalar(out=s_out, in0=v, scalar1=0.5,
                                            scalar2=s_scale, op0=Alu.is_gt,
                                            op1=Alu.mult)
                    if l == 2:
                        nc.vector.tensor_tensor(out=accS[:], in0=accS[:],
                                                in1=s_out, op=Alu.add)
                    return
                A2 = scrA2[:, :h]
                if EXACT_ORDER:
                    # u_t = u0 + ((-0.172*v0) + (0.529*u0))  (reference rounding)
                    # ACT muls + Pool adds (SBUF-only tensor_tensor is the only
                    # elementwise op GPSIMD supports), parallel to the DVE
                    # v-chain below
                    nc.scalar.mul(A, v0[:], -0.172)
                    nc.scalar.mul(A2, u0[:], 0.529)
                    nc.gpsimd.tensor_tensor(out=A2, in0=A, in1=A2, op=Alu.add)
                    nc.gpsimd.tensor_tensor(out=A2, in0=u0[:], in1=A2, op=Alu.add)
                    # dv = ((q - v0) - u0) + c;  v = v0 + dv  (reference rounding)
                    nc.vector.tensor_tensor(out=v, in0=q[:], in1=v0[:],
                                            op=Alu.subtract)
                    nc.vector.tensor_tensor(out=v, in0=v, in1=u0[:],
                                            op=Alu.subtract)
                    add_c()
                    nc.vector.tensor_tensor(out=v, in0=v0[:], in1=v, op=Alu.add)
                else:
                    nc.vector.scalar_tensor_tensor(
                        out=A, in0=v0[:], scalar=float(-0.172 / 1.529), in1=u0[:],
                        op0=Alu.mult, op1=Alu.add)
                    nc.vector.tensor_scalar(out=A, in0=A, scalar1=1.529,
                                            scalar2=None, op0=Alu.mult)
                    nc.vector.tensor_tensor(out=v, in0=q[:], in1=u0[:],
                                            op=Alu.subtract)
                    add_c()
                # spikes (scale 2^t for l<2; unscaled for l==2) -> fp16
                s_scale = 1.0 if l == 2 else float(2.0 ** t)
                nc.vector.tensor_scalar(out=s_out, in0=v, scalar1=0.5,
                                        scalar2=s_scale, op0=Alu.is_gt,
                                        op1=Alu.mult)
                if l == 2:
                    nc.gpsimd.tensor_tensor(out=accS[:], in0=accS[:], in1=s_out,
                                            op=Alu.add)
                # u0_{t+1} = u_t + 0.132 * s_t     (unscale s_out)
                nc.vector.scalar_tensor_tensor(
                    out=u0[:], in0=s_out, scalar=float(0.132 / s_scale),
                    in1=(A2 if EXACT_ORDER else A),
                    op0=Alu.mult, op1=Alu.add)
                # v0_{t+1} = v_t with 0.021 where spiked
                nc.scalar.copy(v0[:], v)
                nc.vector.copy_predicated(out=v0[:], mask=s_out.bitcast(dt.uint16),
                                          data=c021[:, :h])
                # q_{t+1} = v0^2
                nc.scalar.square(q[:], v0[:])

            def matmuls(l, t, h, lhsA, lhsR, nm, k_lo, k_hi, kt_total,
                        bias=None, ones2=None, lhs_base=0):
                """Accumulate 2^t * (x@W + b) into C[l] (+C0b lo-part for l=0).

                Hi-term matmuls are emitted before lo-term ones so the PE
                queue never blocks on the (later-ready) lo operand.
                """
                if bias is not None:
                    # for l>0 this is the first write of step 0 into the bank
                    for n0 in range(0, h, NCH):
                        nn = min(NCH, h - n0)
                        nc.tensor.matmul(C[l][:, n0:n0 + nn], ones2[:],
                                         bias[:, n0:n0 + nn],
                                         start=(t == 0 and l != 0), stop=False,
                                         skip_group_check=True)
                for kg in range(k_lo, k_hi):
                    for n0 in range(0, h, NCH):
                        nn = min(NCH, h - n0)
                        first = (t == 0 and kg == 0 and l == 0)
                        ps = C[l][:, n0:n0 + nn]
                        ra = wsl(nm, "a", kg, h, n0, nn)
                        la = lhsA[:, (kg - lhs_base) * 128:(kg - lhs_base + 1) * 128]
                        nc.tensor.matmul(ps, la, ra, start=first, stop=False,
                                         skip_group_check=True)
                if l == 0:
                    for kg in range(k_lo, k_hi):
                        for n0 in range(0, h, NCH):
                            nn = min(NCH, h - n0)
                            first = (t == 0 and kg == 0)
                            ra = wsl(nm, "a", kg, h, n0, nn)
                            lr = lhsR[:, (kg - lhs_base) * 128:(kg - lhs_base + 1) * 128]
                            nc.tensor.matmul(C0b[:, n0:n0 + nn], lr, ra,
                                             start=first, stop=False,
                                             skip_group_check=True)
                for kg in range(k_lo, k_hi):
                    for n0 in range(0, h, NCH):
                        nn = min(NCH, h - n0)
                        last = (t == T - 1 and kg == kt_total - 1)
                        rl = wsl(nm, "l", kg, h, n0, nn)
                        la = lhsA[:, (kg - lhs_base) * 128:(kg - lhs_base + 1) * 128]
                        if l == 0:
                            nc.tensor.matmul(C0b[:, n0:n0 + nn], la, rl,
                                             start=False, stop=last,
                                             skip_group_check=True)
                        else:
                            lr = lhsR[:, (kg - lhs_base) * 128:(kg - lhs_base + 1) * 128]
                            nc.tensor.matmul(C[l][:, n0:n0 + nn], lr, rl,
                                             start=False, stop=last,
                                             skip_group_check=True)

            ones2_h = {}
            x_pre = {}

            def load_x(t, eng=None):
                eng = eng or nc.sync
                ones2 = xp.tile([2, 128], F16, tag="ones2", name=f"ones2_t{t}")
                nc.gpsimd.memset(ones2[:, :], float(2.0 ** (t - 11)))
                nc.gpsimd.memset(ones2[0:1, :], float(2.0 ** t))
                ones2_h[t] = ones2
                xa_t = xp.tile([128, KT0 * BL], F16, tag="xa", name=f"xa_t{t}")
                xr_t = xp.tile([128, KT0 * BL], F16, tag="xr", name=f"xr_t{t}")
                eng.dma_start(
                    out=xa_t[:], in_=xa_d[t:t + 1].rearrange("o p f -> (o p) f"))
                eng.dma_start(
                    out=xr_t[:], in_=xr_d[t:t + 1].rearrange("o p f -> (o p) f"))
                x_pre[t] = (xa_t, xr_t)

            NX0 = 2
            KH = KT0 // NX0

            def emit_L0(t, cis):
                xa_t, xr_t = x_pre[t]
                if 1 in cis:
                    x_pre.pop(t, None)
                for ci in cis:
                    matmuls(0, t, H0, xa_t[:], xr_t[:], "w0",
                            ci * KH, (ci + 1) * KH, KT0,
                            bias=b_sb["b0"] if ci == NX0 - 1 else None,
                            ones2=ones2_h[t])

            def lif_chain_halves(l, t, s_tile, last=False, v_base=None,
                                 c_src=None):
                """v-chain + spike for layer l in two half-width slices; each
                half is immediately DMA-transposed into its own tile (so the
                next layer's first matmul half starts as early as possible).
                Returns [(sTh, sLh), (sTh, sLh)]."""
                h = HS[l]
                u0, v0, q = (st[(l, n)] for n in ("u0", "v0", "q"))
                halves = []
                for hf in (0, 1):
                    sl = slice(hf * (h // 2), (hf + 1) * (h // 2))
                    v = (v_base if v_base is not None else scrV)[:, sl]
                    nc.vector.tensor_tensor(out=v, in0=q[:, sl], in1=v0[:, sl],
                                            op=Alu.subtract)
                    nc.vector.tensor_tensor(out=v, in0=v, in1=u0[:, sl],
                                            op=Alu.subtract)
                    if l == 0:
                        nc.vector.tensor_tensor(out=v, in0=v,
                                                in1=scrB0b[:, sl], op=Alu.add)
                        nc.vector.tensor_tensor(out=v, in0=v, in1=scrB0[:, sl],
                                                op=Alu.add)
                    elif c_src is not None:
                        nc.vector.tensor_tensor(out=v, in0=v, in1=c_src[:, sl],
                                                op=Alu.add)
                    else:
                        nc.vector.scalar_tensor_tensor(
                            out=v, in0=C[l][:, sl], scalar=float(2.0 ** -t),
                            in1=v, op0=Alu.mult, op1=Alu.add)
                    nc.vector.tensor_tensor(out=v, in0=v0[:, sl], in1=v,
                                            op=Alu.add)
                    nc.vector.tensor_scalar(out=s_tile[:, sl], in0=v,
                                            scalar1=0.5,
                                            scalar2=float(2.0 ** t),
                                            op0=Alu.is_gt, op1=Alu.mult)
                    sTh = kp.tile([128, h // 2], F16, tag="sTh",
                                  name=f"sT{l}_t{t}_h{hf}", bufs=4)
                    teng = nc.scalar if l == 0 else nc.sync
                    teng.dma_start_transpose(
                        out=sTh[:].rearrange("p (k b) -> p k b", b=128),
                        in_=s_tile[:, sl])
                    sLh = kp.tile([128, h // 2], F16, tag="sLh",
                                  name=f"sL{l}_t{t}_h{hf}", bufs=4)
                    nc.vector.tensor_scalar(out=sLh[:], in0=sTh[:],
                                            scalar1=float(2.0 ** -11),
                                            scalar2=None, op0=Alu.mult)
                    halves.append((sTh, sLh))
                return halves

            def lif_post(l, t, s_tile, v_base=None):
                """state updates for t+1 (full width, off the spike path)."""
                h = HS[l]
                u0, v0, q = (st[(l, n)] for n in ("u0", "v0", "q"))
                v = (v_base if v_base is not None else scrV)[:, :h]
                A2 = scrA2[:, :h]
                s_scale = float(2.0 ** t)
                nc.vector.scalar_tensor_tensor(
                    out=u0[:], in0=s_tile[:], scalar=float(0.132 / s_scale),
                    in1=A2, op0=Alu.mult, op1=Alu.add)
                nc.scalar.copy(v0[:], v)
                nc.vector.copy_predicated(
                    out=v0[:], mask=s_tile[:].bitcast(dt.uint16),
                    data=c021[:, :h])
                nc.scalar.square(q[:], v0[:])

            def u_subchain(l):
                """u_t = u0 + ((-0.172*v0) + (0.529*u0)), reference rounding;
                ACT muls + Pool adds, parallel to the DVE v-chain."""
                h = HS[l]
                u0, v0 = st[(l, "u0")], st[(l, "v0")]
                A = scrA[:, :h]
                A2 = scrA2[:, :h]
                nc.scalar.mul(A, v0[:], -0.172)
                nc.scalar.mul(A2, u0[:], 0.529)
                nc.gpsimd.tensor_tensor(out=A2, in0=A, in1=A2, op=Alu.add)
                nc.gpsimd.tensor_tensor(out=A2, in0=u0[:], in1=A2, op=Alu.add)

            def matmuls_next(l, t, h, nm, kt, halves, bias):
                """next-layer matmuls from spike halves: bias, then per half
                hi then lo."""
                for n0 in range(0, h, NCH):
                    nn = min(NCH, h - n0)
                    nc.tensor.matmul(C[l][:, n0:n0 + nn], ones2_h[t][:],
                                     bias[:, n0:n0 + nn],
                                     start=(t == 0), stop=False,
                                     skip_group_check=True)
                kh = kt // 2
                for hf in (0, 1):
                    sTh, sLh = halves[hf]
                    matmuls(l, t, h, sTh[:], sLh[:], nm,
                            hf * kh, (hf + 1) * kh, kt,
                            lhs_base=hf * kh)

            def emit_l0_spike(t, v_tile=None):
                s0 = kp.tile([128, H0], F16, tag="sPre", name=f"s0_t{t}",
                             bufs=2)
                last = (t == T - 1)
                if not last:
                    u_subchain(0)
                halves = lif_chain_halves(0, t, s0, last=last, v_base=v_tile)
                if not last:
                    lif_post(0, t, s0, v_base=v_tile)
                return halves

            def emit_L1(t, halves):
                if t == 2:
                    warm(12)
                elif t == 14:
                    warm(13)
                elif t == 15:
                    warm(22)
                matmuls_next(1, t, H1, "w1", KT1, halves, b_sb["b1"])

            tail_l1 = []

            def emit_rest(t, filler=None, skip_l1=False):
                if not skip_l1:
                    halves = emit_l0_spike(t)
                    emit_L1(t, halves)
                c1_src = None
                if t == T - 2:
                    # the filler below hoists L1(T-1) into C1: release C1 for
                    # step t first (exact power-of-two scale)
                    nc.scalar.mul(scrB1[:], C[1][:], float(2.0 ** -t))
                    c1_src = scrB1
                if filler is not None:
                    filler()
                last = (t == T - 1)
                if last:
                    halves1 = tail_l1[0]
                else:
                    s1 = kp.tile([128, H1], F16, tag="sPre", name=f"s1_t{t}",
                                 bufs=2)
                    u_subchain(1)
                    halves1 = lif_chain_halves(1, t, s1, last=False,
                                               c_src=c1_src)
                    lif_post(1, t, s1)
                if t >= 14:
                    warm(10 if t < T - 1 else 18)
                matmuls_next(2, t, OUT, "wo", KT2, halves1, b_sb["b2"])
                if t == T - 2:
                    # C1 holds all 16 steps now (L1(15) was hoisted into the
                    # filler): emit the t=15 l1 chain + transposes early, so
                    # L2(15)'s operands are ready long before the PE drains
                    s1f = kp.tile([128, H1], F16, tag="sPre",
                                  name=f"s1_t{T - 1}", bufs=2)
                    tail_l1.append(
                        lif_chain_halves(1, T - 1, s1f, last=True))
                s2 = kp.tile([128, OUT], F16, tag="s2", name=f"s2_t{t}", bufs=1)
                if not last:
                    lif_ops(2, t, s2[:], last=False)
                else:
                    # final drain: half-width chain, acc and output DMA per
                    # half so the first out-DMA starts early
                    u0, v0, q = (st[(2, n)] for n in ("u0", "v0", "q"))
                    for hf in (0, 1):
                        sl = slice(hf * (OUT // 2), (hf + 1) * (OUT // 2))
                        v = scrV[:, sl]
                        nc.vector.tensor_tensor(out=v, in0=q[:, sl],
                                                in1=v0[:, sl], op=Alu.subtract)
                        nc.vector.tensor_tensor(out=v, in0=v, in1=u0[:, sl],
                                                op=Alu.subtract)
                        nc.vector.scalar_tensor_tensor(
                            out=v, in0=C[2][:, sl], scalar=float(2.0 ** -t),
                            in1=v, op0=Alu.mult, op1=Alu.add)
                        nc.vector.tensor_tensor(out=v, in0=v0[:, sl], in1=v,
                                                op=Alu.add)
                        nc.vector.tensor_scalar(out=s2[:, sl], in0=v,
                                                scalar1=0.5, scalar2=1.0,
                                                op0=Alu.is_gt, op1=Alu.mult)
                        nc.vector.tensor_tensor(out=accS[:, sl],
                                                in0=accS[:, sl],
                                                in1=s2[:, sl], op=Alu.add)
                        nc.sync.dma_start(out=out_d[:, sl], in_=accS[:, sl])
                ones2_h.pop(t, None)

            # preamble DMAs: the single serial DMA engine makes this order the
            # startup schedule. x(0) first, then w0 (a/l interleaved by group,
            # with biases tucked in), x(1), w1, wo.
            ones2_0 = xp.tile([2, 128], F16, tag="ones2", name="ones2_t0")
            nc.gpsimd.memset(ones2_0[:, :], float(2.0 ** -11))
            nc.gpsimd.memset(ones2_0[0:1, :], 1.0)
            ones2_h[0] = ones2_0
            xa_0 = xp.tile([128, KT0 * BL], F16, tag="xa", name="xa_t0")
            xr_0 = xp.tile([128, KT0 * BL], F16, tag="xr", name="xr_t0")
            nc.sync.dma_start(
                out=xa_0[:], in_=xa_d[0:1].rearrange("o p f -> (o p) f"))
            x_pre[0] = (xa_0, xr_0)
            for _ in range(12):
                nc.tensor.matmul(dummyP[:, :128], ones2_0[:], ones2_0[:],
                                 start=True, stop=True, skip_group_check=True)
            dma_w("w0", "a", 0)
            dma_w("w0", "a", 1)
            nc.sync.dma_start(
                out=xr_0[:], in_=xr_d[0:1].rearrange("o p f -> (o p) f"))
            dma_w("w0", "a", 2)
            dma_w("w0", "a", 3)
            dma_w("w0", "l", 0)
            dma_w("w0", "l", 1)
            dma_b("b0")
            dma_w("w0", "l", 2)
            dma_w("w0", "l", 3)
            for g in range(4, KT0 // GK["w0"]):
                dma_w("w0", "a", g)
            for g in range(4, KT0 // GK["w0"]):
                dma_w("w0", "l", g)
            load_x(1)
            dma_w("w1", "a", 0)
            dma_b("b1")
            dma_w("w1", "a", 1)
            dma_w("w1", "l", 0)
            dma_w("w1", "l", 1)
            dma_b("b2")
            dma_w("wo", "a", 0)
            dma_w("wo", "l", 0)

            # 1-step layer skew: PE gets L0(t+1) while the t chain drains
            def tail_filler(tt):
                emit_L0(tt, cis=(1,))
                if tt == T - 1:
                    # t=15 l0 chain has no state carry: independent of the
                    # t=14 l1/l2 chains -> emit now so it runs early on DVE
                    # and L1(15) lands on the PE right after L0(15).
                    lif_B(0, tt)
                    emit_L1(tt, emit_l0_spike(tt))

            for t in range(T):
                if t >= 1:
                    lif_B(0, t - 1)       # free C0/C0b for step t's matmuls
                emit_L0(t, cis=(0,))
                if t >= 1:
                    emit_rest(t - 1, filler=lambda tt=t: tail_filler(tt))
                else:
                    emit_L0(t, cis=(1,))
                # issue x(t+1) after the step's transposes so they win the
                # (serial) DMA engine; plenty of slack before it's needed
                if t + 1 < T and t + 1 not in x_pre:
                    load_x(t + 1)
            emit_rest(T - 1, skip_l1=True)

    nc.compile()
    _BUILD_CACHE[key] = nc
    return nc


def _split_f16(a32, lo_scale=2048.0):
    """a32 ~ hi + lo*2^-11 with hi = fp16(a32), lo = fp16((a32-hi)*2^11)."""
    hi = a32.astype(np.float16)
    lo = ((a32 - hi.astype(np.float32)) * np.float32(lo_scale)).astype(np.float16)
    return hi, lo


def _pmajor(w, kt, h):
    """[kt*128, h] -> [128, kt*h] partition-major blocks."""
    return np.ascontiguousarray(
        w.reshape(kt, 128, h).transpose(1, 0, 2).reshape(128, kt * h))


def prep_inputs(in_pop_spikes, W0, b0, W1, b1, Wout, bout,
                T=16, BL=128, ncores=NCORES):
    """Host-side prep: transpose/scale/split x, split weights; 8 in_maps."""
    x = np.ascontiguousarray(np.transpose(np.asarray(in_pop_spikes, np.float32),
                                          (2, 1, 0)))  # [T, IN, B]
    TT, IN, B = x.shape
    KT0 = IN // 128
    scale = (2.0 ** np.arange(T, dtype=np.float32)).reshape(T, 1, 1)
    xh32 = x.astype(np.float16).astype(np.float32)
    xa = (xh32 * scale).astype(np.float16)                 # exact 2^t * fp16(x)
    xr = ((x - xh32) * (scale * np.float32(2048.0))).astype(np.float16)
    # ^ 2^(t+11) * xl, fp16 (xl itself is the exact fp32 residual)

    com = {}
    for nm, W in (("w0", W0), ("w1", W1), ("wo", Wout)):
        WT = np.ascontiguousarray(np.asarray(W, np.float32).T)
        kt, h = WT.shape[0] // 128, WT.shape[1]
        hi, lo = _split_f16(WT)
        com[nm + "a"] = _pmajor(hi, kt, h)
        com[nm + "l"] = _pmajor(lo, kt, h)
    for nm, b in (("b0", b0), ("b1", b1), ("b2", bout)):
        hi, lo = _split_f16(np.asarray(b, np.float32))
        com[nm] = np.stack([hi, lo])

    in_maps = []
    for c in range(ncores):
        m = dict(com)
        # [T, IN, BL] -> [T, 128, KT0*BL] partition-major
        for nm, arr in (("xa", xa), ("xr", xr)):
            sl = arr[:, :, c * BL:(c + 1) * BL]
            m[nm] = np.ascontiguousarray(
                sl.reshape(T, KT0, 128, BL).transpose(0, 2, 1, 3)
                .reshape(T, 128, KT0 * BL))
        in_maps.append(m)
    return in_maps


def kernel(in_pop_spikes, W0, b0, W1, b1, Wout, bout, batch_size, _trace=False):
    T = in_pop_spikes.shape[2]
    nc = build(**FULL)
    in_maps = prep_inputs(in_pop_spikes, W0, b0, W1, b1, Wout, bout, T=T)
    res = run_bass_kernel_spmd(nc, in_maps, core_ids=list(range(NCORES)),
                               trace=_trace)
    out = np.concatenate([r["out"] for r in res.results], axis=0)
    out = (out / np.float32(T)).astype(np.float32)
    if _trace:
        kernel._last_results = res
    return out
